# revision 1
# baseline (speedup 1.0000x reference)
"""Biased MF batch scoring on 8 NeuronCores — tuned indirect-gather kernel.

Same combined-table structure as the 58us baseline (32 single-offset
indirect_dma_start ops on the Pool SWDGE; multi-offset ops are broken on HW:
they stream a contiguous run from the first offset), with:
  - fp16 table rows (132B): halves descriptor payload (drain time) + 2x DVE.
  - uneven chunks [8,8,8,6,2]: the last compute chunk covers 2 ops (128
    elements) so the tail (last DMA completion + mul + reduce + store) is
    small.
  - output stores split: cols for chunks 0-3 store while chunk 4 computes.

Layout: idx column k in [0,32): op k gathers row idx[p, k] into
rows[:, k*W:(k+1)*W].  Columns 2c*G..: chunk c has CH[c] ops: first half user
rows, second half item rows of the chunk's elements.
user row = [uf(64)|ub|1], item row = [itf(64)|1|ib+3.5] (fp16, W=66): the
row-pair dot is the final answer.
"""

import numpy as np

GLOBAL_AVERAGE = 3.5
NUM_USERS = 1_000_000
NUM_ITEMS = 100_000
F = 64
B = 16384
NCORES = 8
BC = B // NCORES
P = 128
EPP = BC // P  # 16 elements per partition
W = F + 2  # 66
CH = [8, 8, 8, 6, 2]  # ops per chunk (each chunk: CH/2 user ops + CH/2 item)
NCH = len(CH)

TRACE = False
LAST_RES = None
_BUILD_CACHE = {}


def build_nc():
    if 0 in _BUILD_CACHE:
        return _BUILD_CACHE[0]
    import concourse.bass as bass
    import concourse.mybir as mybir
    from concourse.bass import IndirectOffsetOnAxis
    from contextlib import ExitStack

    ncat = NUM_USERS + NUM_ITEMS
    nc = bass.Bass(num_swdge_queues=2)
    idx = nc.dram_tensor("idx", [P, 2 * EPP], mybir.dt.int32, kind="ExternalInput")
    cat = nc.dram_tensor("cat", [ncat, W], mybir.dt.float16, kind="ExternalInput")
    out = nc.dram_tensor("out", [P, EPP], mybir.dt.float32, kind="ExternalOutput")

    with ExitStack() as stack:
        e = stack.enter_context
        t_idx = e(nc.sbuf_tensor("t_idx", [P, 2 * EPP], mybir.dt.int32))
        rows = e(nc.sbuf_tensor("rows", [P, 2 * EPP * W], mybir.dt.float16))
        prod = e(nc.sbuf_tensor("prod", [P, EPP * W], mybir.dt.float16))
        res = e(nc.sbuf_tensor("res", [P, EPP], mybir.dt.float32))
        s_idx = e(nc.semaphore("s_idx"))
        s_g = [e(nc.semaphore(f"s_g{c}")) for c in range(NCH)]
        s_v = e(nc.semaphore("s_v"))
        s_c = e(nc.semaphore("s_c"))
        s_o = e(nc.semaphore("s_o"))
        block = e(nc.Block())

        # chunk -> (op offset, element-column offset, #element-cols)
        op_off = [sum(CH[:c]) for c in range(NCH)]
        ecols = [ch // 2 for ch in CH]
        ecol_off = [sum(ecols[:c]) for c in range(NCH)]

        @block.sync
        def _(sy):
            sy.dma_start(t_idx[:], idx[:]).then_inc(s_idx, 16)
            sy.wait_ge(s_c, NCH - 1)
            sy.dma_start(
                out[:, : ecol_off[NCH - 1]], res[:, : ecol_off[NCH - 1]]
            ).then_inc(s_o, 16)
            sy.wait_ge(s_c, NCH)
            with nc.allow_non_contiguous_dma(reason="single trailing column"):
                sy.dma_start(
                    out[:, ecol_off[NCH - 1] :], res[:, ecol_off[NCH - 1] :]
                ).then_inc(s_o, 16)
            sy.wait_ge(s_o, 32)

        @block.gpsimd
        def _(g):
            g.wait_ge(s_idx, 16)
            for c in range(NCH):
                for j in range(CH[c]):
                    k = op_off[c] + j
                    g.indirect_dma_start(
                        out=rows[:, k * W : (k + 1) * W],
                        out_offset=None,
                        in_=cat[:],
                        in_offset=IndirectOffsetOnAxis(
                            ap=t_idx[:, k : k + 1], axis=0
                        ),
                    ).then_inc(s_g[c], 16)

        @block.vector
        def _(vec):
            for c in range(NCH):
                h = CH[c] // 2
                lo = op_off[c] * W
                po = ecol_off[c] * W
                vec.wait_ge(s_g[c], 16 * CH[c])
                vec.tensor_mul(
                    prod[:, po : po + h * W],
                    rows[:, lo : lo + h * W],
                    rows[:, lo + h * W : lo + 2 * h * W],
                ).then_inc(s_v, 1)
                vec.wait_ge(s_v, c + 1)
                vec.reduce_sum(
                    res[:, ecol_off[c] : ecol_off[c] + h],
                    prod[:, po : po + h * W].rearrange("p (g w) -> p g w", w=W),
                    axis=mybir.AxisListType.X,
                ).then_inc(s_c, 1)

    nc.finalize()
    _strip_boot_barrier(nc)
    # alternate gather ops across the two SWDGE queues (separate desc rings);
    # per-op completion sems make chunk waits queue-order-independent
    k = 0
    for bb in nc.m.functions[0].blocks:
        for ins in bb.instructions:
            if type(ins).__name__ == "InstDMACopy" and getattr(ins, "queue", "") == "qPoolDynamic":
                if k % 2 == 1:
                    ins.queue = "qPoolDynamic1"
                k += 1
    assert k == 32, k
    _BUILD_CACHE[0] = nc
    return nc


def _strip_boot_barrier(nc):
    barrier_sem_ids = set()
    for bb in nc.m.functions[0].blocks:
        for ins in bb.instructions:
            si = ins.sync_info
            if si:
                for u in list(si.on_update or []) + list(si.on_wait or []):
                    if "barrier_" in (getattr(u, "ant_name", "") or ""):
                        barrier_sem_ids.add(u.id)
    for bb in nc.m.functions[0].blocks:
        if bb.name != "main":
            continue
        keep = []
        for ins in bb.instructions:
            tn = type(ins).__name__
            drop = tn == "InstMemset"
            si = ins.sync_info
            if not drop and si and tn in ("InstDrain", "InstEventSemaphore"):
                drop = any(
                    getattr(u, "id", None) in barrier_sem_ids
                    for u in list(si.on_update or []) + list(si.on_wait or [])
                )
            if not drop:
                keep.append(ins)
        if len(keep) != len(bb.instructions):
            bb.instructions[:] = keep
    used = set()
    for bb in nc.m.functions[0].blocks:
        for ins in bb.instructions:
            si = ins.sync_info
            if si:
                for u in list(si.on_update or []) + list(si.on_wait or []):
                    sid = getattr(u, "id", None)
                    if sid is not None:
                        used.add(sid)
    for bb in nc.m.functions[0].blocks:
        keep = []
        for ins in bb.instructions:
            drop = False
            if type(ins).__name__ == "InstEventSemaphore":
                si = ins.sync_info
                ups = list(si.on_update or []) if si else []
                ws = list(si.on_wait or []) if si else []
                if not ws and len(ups) == 1:
                    u = ups[0]
                    if (
                        getattr(u, "value", None) == 0
                        and getattr(u, "sem_op", None) in ("set", "assign", None)
                        and getattr(u, "id", -1) not in used
                    ):
                        drop = True
            if not drop:
                keep.append(ins)
        if len(keep) != len(bb.instructions):
            bb.instructions[:] = keep


def make_cat(user_factors, item_factors, user_biases, item_biases):
    nu, f = user_factors.shape
    ni = item_factors.shape[0]
    cat = np.empty((nu + ni, W), np.float16)
    cat[:nu, :f] = user_factors
    cat[:nu, f] = np.asarray(user_biases).reshape(nu)
    cat[:nu, f + 1] = 1.0
    cat[nu:, :f] = item_factors
    cat[nu:, f] = 1.0
    cat[nu:, f + 1] = np.asarray(item_biases).reshape(ni) + np.float32(GLOBAL_AVERAGE)
    return cat


def make_idx(users, items):
    """Element (core, p, e) = batch index core*BC + e*P + p, e in [0,16).
    Chunk c covers elements e in [ecol_off[c], ecol_off[c]+CH[c]//2); its user
    ops are idx columns op_off[c]..+CH[c]//2-1 (element order), item ops next.
    """
    u = np.asarray(users, dtype=np.int32).reshape(NCORES, EPP, P)
    it = np.asarray(items, dtype=np.int32).reshape(NCORES, EPP, P) + np.int32(NUM_USERS)
    out = np.empty((NCORES, P, 2 * EPP), np.int32)
    op_off = [sum(CH[:c]) for c in range(NCH)]
    ecols = [ch // 2 for ch in CH]
    ecol_off = [sum(ecols[:c]) for c in range(NCH)]
    for c in range(NCH):
        h = ecols[c]
        esl = slice(ecol_off[c], ecol_off[c] + h)
        out[:, :, op_off[c] : op_off[c] + h] = u[:, esl, :].transpose(0, 2, 1)
        out[:, :, op_off[c] + h : op_off[c] + 2 * h] = it[:, esl, :].transpose(0, 2, 1)
    return out


def kernel(users, items, user_factors, item_factors, user_biases, item_biases):
    global LAST_RES
    from concourse.bass_utils import run_bass_kernel_spmd

    nc = build_nc()
    cat = make_cat(user_factors, item_factors, user_biases, item_biases)
    idx = make_idx(users, items)
    in_maps = [{"idx": idx[c], "cat": cat} for c in range(NCORES)]
    res = run_bass_kernel_spmd(nc, in_maps, core_ids=list(range(NCORES)), trace=TRACE)
    LAST_RES = res
    outs = []
    for c in range(NCORES):
        o = res.results[c]["out"]  # [P, EPP]; element (p, e) = c*BC + e*P + p
        outs.append(o.T.reshape(-1))
    return np.concatenate(outs).astype(np.float32)



# revision 2
# speedup vs baseline: 1.0023x; 1.0023x over previous
"""Biased MF batch scoring on 8 NeuronCores — async multi-queue dma_gather.

Key empirical rules (measured on this HW):
  - dma_gather on SWDGE queue 0 BLOCKS the GpSimd engine ~10ns/idx; on
    queues 1-3 it is fire-and-forget (~94ns dispatch), desc-gen runs on the
    queue's own Q7 cpu pair and the completion sem fires on DMA completion.
    => put all gathers on q1-q3, nothing on q0.
  - DVE rates: tensor_tensor ~0.6ns/elem/partition, tensor_reduce ~1.1.
    => slim item srows (4 x 66 f16 chunks in 768B) to cut compute 2x vs
    128-elem chunks, and chunk compute per bank block to overlap gathers.

Structure per core (elements assigned to cores by user-id range):
  - 4 user bank gathers (bank = 32768 ids, int16 idx), 256B rows, q3.
  - 4 item slot-chunk gathers from the k=4 packed slim table
    (25000 srows x 768B; srow j holds rows 4j..4j+3 as 66-f16 slices),
    alternating q1/q2. Slot order == user arrival order; the one-hot
    residue mask folds the 4-candidate dots into the final answer.
  - Vector: per block b: prod = irows*u(bcast), reduce 66 -> dots4;
    at the end: res = reduce_4(dots4 * mask).
"""

import numpy as np

GLOBAL_AVERAGE = 3.5
NU = 1_000_000
NI = 100_000
F = 64
B = 16384
NCORES = 8
USHARD = 131072
UBANK = 32768
NBANKS = 4
ISROWS = 25000
W = 128  # user row elements (fp16, 256B)
IW = 66  # slim item row elements
ISW = 4 * 96  # item srow elements (4*96 f16 = 768B; chunks at 66-elem strides padded)

TRACE = False
LAST_RES = None
_BUILD_CACHE = {}


def build_nc(cap):
    key = ("v6", cap)
    if key in _BUILD_CACHE:
        return _BUILD_CACHE[key]
    import concourse.bass as bass
    import concourse.mybir as mybir
    from concourse.ap import AP
    from concourse.library_config import mlp
    from contextlib import ExitStack

    S = NBANKS * cap
    C = S // 128
    cc = cap // 128

    nc = bass.Bass(num_swdge_queues=4)
    ucat = nc.dram_tensor("ucat", [USHARD, W], mybir.dt.float16, kind="ExternalInput")
    icat = nc.dram_tensor("icat", [ISROWS, ISW], mybir.dt.float16, kind="ExternalInput")
    uidx = nc.dram_tensor("uidx", [128, S // 16], mybir.dt.int16, kind="ExternalInput")
    iidx = nc.dram_tensor("iidx", [128, S // 16], mybir.dt.int16, kind="ExternalInput")
    mask = nc.dram_tensor("mask", [128, C * 4], mybir.dt.float32, kind="ExternalInput")
    didx = nc.dram_tensor("didx", [128, 1], mybir.dt.int16, kind="ExternalInput")
    out = nc.dram_tensor("out", [128, C], mybir.dt.float32, kind="ExternalOutput")

    with ExitStack() as stack:
        e = stack.enter_context
        t_uidx = e(nc.sbuf_tensor("t_uidx", [128, S // 16], mybir.dt.int16))
        t_iidx = e(nc.sbuf_tensor("t_iidx", [128, S // 16], mybir.dt.int16))
        t_mask = e(nc.sbuf_tensor("t_mask", [128, C * 4], mybir.dt.float32))
        t_didx = e(nc.sbuf_tensor("t_didx", [128, 1], mybir.dt.int16))
        dscr = e(nc.sbuf_tensor("dscr", [128, ISW], mybir.dt.float16))
        urows = e(nc.sbuf_tensor("urows", [128, C * W], mybir.dt.float16))
        irows = e(nc.sbuf_tensor("irows", [128, C * ISW], mybir.dt.float16))
        prod = e(nc.sbuf_tensor("prod", [128, C * 4 * IW], mybir.dt.float16))
        dots4 = e(nc.sbuf_tensor("dots4", [128, C * 4], mybir.dt.float32))
        resm = e(nc.sbuf_tensor("resm", [128, C * 4], mybir.dt.float32))
        res = e(nc.sbuf_tensor("res", [128, C], mybir.dt.float32))
        s_idx = e(nc.semaphore("s_idx"))
        s_m = e(nc.semaphore("s_m"))
        s_iblk = [e(nc.semaphore(f"s_ib{b}")) for b in range(NBANKS)]
        s_ublk = [e(nc.semaphore(f"s_ub{b}")) for b in range(NBANKS)]
        s_d = e(nc.semaphore("s_d"))
        s_dg = e(nc.semaphore("s_dg"))
        s_v = e(nc.semaphore("s_v"))
        s_o = e(nc.semaphore("s_o"))
        block = e(nc.Block())

        @block.sync
        def _(sy):
            sy.dma_start(t_didx[:], didx[:]).then_inc(s_d, 16)
            sy.dma_start(t_iidx[:], iidx[:]).then_inc(s_idx, 16)
            sy.dma_start(t_uidx[:], uidx[:]).then_inc(s_idx, 16)
            sy.dma_start(t_mask[:], mask[:]).then_inc(s_m, 16)
            sy.wait_ge(s_v, 2 * NBANKS + 2)
            sy.dma_start(out[:], res[:]).then_inc(s_o, 16)
            sy.wait_ge(s_o, 16)

        @block.gpsimd
        def _(g):
            g.load_library(mlp)
            g.wait_ge(s_d, 16)
            # tiny gather: absorbs the post-reload dispatch block on its own
            # queue so the real calls dispatch fire-and-forget
            g.dma_gather(
                out_ap=dscr[:].rearrange("p (c k) -> p c k", k=ISW),
                in_ap=icat[:],
                idxs_ap=t_didx[:],
                num_idxs=16,
                num_idxs_reg=16,
                elem_size=ISW,
                queue_num=3,
                single_packet=False,
            ).then_inc(s_dg, 16)
            g.wait_ge(s_idx, 32)
            nreg = g.to_reg(cap)
            iv = irows[:].rearrange("p (c k) -> p c k", k=ISW)
            uv = urows[:].rearrange("p (c k) -> p c k", k=W)
            def icall(b, q):
                g.dma_gather(
                    out_ap=iv[:, b * cc : (b + 1) * cc, :],
                    in_ap=icat[:],
                    idxs_ap=t_iidx[:, b * (cap // 16) : (b + 1) * (cap // 16)],
                    num_idxs=cap,
                    num_idxs_reg=nreg,
                    elem_size=ISW,
                    queue_num=q,
                    single_packet=False,
                ).then_inc(s_iblk[b], 16)

            def ucall(b, q):
                g.dma_gather(
                    out_ap=uv[:, b * cc : (b + 1) * cc, :],
                    in_ap=ucat[b * UBANK : (b + 1) * UBANK, :],
                    idxs_ap=t_uidx[:, b * (cap // 16) : (b + 1) * (cap // 16)],
                    num_idxs=cap,
                    num_idxs_reg=nreg,
                    elem_size=W,
                    queue_num=q,
                    single_packet=False,
                ).then_inc(s_ublk[b], 16)

            # block-major pairs so each compute block completes ASAP;
            # queues chosen so consecutive calls never share a queue and
            # per-queue load is 3/3/2 (dummy occupies q3 first)
            icall(0, 1)
            ucall(0, 2)
            icall(1, 3)
            ucall(1, 1)
            icall(2, 2)
            ucall(2, 3)
            icall(3, 1)
            ucall(3, 2)

        @block.vector
        def _(vec):
            nv = 0
            for b in range(NBANKS):
                vec.wait_ge(s_iblk[b], 16)
                vec.wait_ge(s_ublk[b], 16)
                # prod[b] = irows chunks (4 x 66) * u (bcast over 4)
                ivb = irows[:].rearrange("p (c k) -> p c k", k=ISW)[
                    :, b * cc : (b + 1) * cc, :
                ]
                i4 = AP(
                    ivb.tensor,
                    ivb.offset,
                    [list(ivb.ap[0]), list(ivb.ap[1]), [96, 4], [1, IW]],
                )
                uvb = urows[:].rearrange("p (c k) -> p c k", k=W)[
                    :, b * cc : (b + 1) * cc, :
                ]
                u4 = AP(
                    uvb.tensor,
                    uvb.offset,
                    [list(uvb.ap[0]), list(uvb.ap[1]), [0, 4], [1, IW]],
                )
                p4 = prod[:].rearrange("p (c r k) -> p c r k", r=4, k=IW)[
                    :, b * cc : (b + 1) * cc, :, :
                ]
                vec.tensor_mul(p4, i4, u4).then_inc(s_v, 1)
                nv += 1
                vec.wait_ge(s_v, nv)
                vec.reduce_sum(
                    dots4[:, b * cc * 4 * 128 // 128 : (b + 1) * cc * 4 * 128 // 128],
                    prod[:, b * cc * 4 * IW * 128 // 128 : (b + 1) * cc * 4 * IW * 128 // 128]
                    .rearrange("p (g k) -> p g k", k=IW),
                    axis=mybir.AxisListType.X,
                ).then_inc(s_v, 1)
                nv += 1
                vec.wait_ge(s_v, nv)
            vec.wait_ge(s_m, 16)
            vec.tensor_mul(resm[:], dots4[:], t_mask[:]).then_inc(s_v, 1)
            nv += 1
            vec.wait_ge(s_v, nv)
            vec.reduce_sum(
                res[:],
                resm[:].rearrange("p (c r) -> p c r", r=4),
                axis=mybir.AxisListType.X,
            ).then_inc(s_v, 1)

    nc.finalize()
    mybir.codegen_inst_isa_subclasses(nc)
    _BUILD_CACHE[key] = nc
    return nc


def _wrap16(vals):
    n = len(vals)
    assert n % 16 == 0
    w = np.asarray(vals, np.int16).reshape(n // 16, 16).T
    return np.tile(w, (8, 1))


def make_inputs(users, items, user_factors, item_factors, user_biases, item_biases):
    users = np.asarray(users, np.int64)
    items = np.asarray(items, np.int64)

    ucat = np.zeros((NCORES * USHARD, W), np.float16)
    ucat[:NU, :F] = user_factors
    ucat[:NU, F] = np.asarray(user_biases, np.float32).reshape(NU)
    ucat[:NU, F + 1] = 1.0
    # slim item rows: [itf(64) | 1 | ib+3.5] = 66 f16, packed 4/srow at 96-elem strides
    irow = np.zeros((NI, IW), np.float16)
    irow[:, :F] = item_factors
    irow[:, F] = 1.0
    irow[:, F + 1] = np.asarray(item_biases, np.float32).reshape(NI) + np.float32(
        GLOBAL_AVERAGE
    )
    icat = np.zeros((ISROWS, ISW), np.float16)
    for r in range(4):
        icat[:, r * 96 : r * 96 + IW] = irow[r::4]
    ushards = ucat.reshape(NCORES, USHARD, W)

    core_of = users >> 17
    bank_of = (users >> 15) & 3
    els = [
        [np.nonzero((core_of == c) & (bank_of == b))[0] for b in range(NBANKS)]
        for c in range(NCORES)
    ]
    maxn = max(len(e) for per in els for e in per)
    cap = -(-maxn // 128) * 128
    S = NBANKS * cap
    C = S // 128

    in_maps = []
    el_of_slot = np.full((NCORES, S), -1, np.int64)
    for c in range(NCORES):
        uidx = np.zeros(S, np.int16)
        iidx = np.zeros(S, np.int16)
        msk = np.zeros((S, 4), np.float32)
        for b in range(NBANKS):
            e = els[c][b]
            n = len(e)
            base = b * cap
            el_of_slot[c, base : base + n] = e
            uidx[base : base + n] = (users[e] & 32767).astype(np.int16)
            iidx[base : base + n] = (items[e] >> 2).astype(np.int16)
            msk[base + np.arange(n), (items[e] & 3)] = 1.0
        uidx_w = np.concatenate(
            [_wrap16(uidx[b * cap : (b + 1) * cap]) for b in range(NBANKS)], axis=1
        )
        iidx_w = np.concatenate(
            [_wrap16(iidx[b * cap : (b + 1) * cap]) for b in range(NBANKS)], axis=1
        )
        mask_t = np.ascontiguousarray(
            msk.reshape(C, 128, 4).transpose(1, 0, 2).reshape(128, C * 4)
        )
        in_maps.append(
            {
                "ucat": ushards[c],
                "icat": icat,
                "uidx": np.ascontiguousarray(uidx_w),
                "iidx": np.ascontiguousarray(iidx_w),
                "mask": mask_t,
                "didx": np.zeros((128, 1), np.int16),
            }
        )
    return cap, in_maps, el_of_slot


def kernel(users, items, user_factors, item_factors, user_biases, item_biases):
    global LAST_RES
    from concourse.bass_utils import run_bass_kernel_spmd

    cap, in_maps, el_of_slot = make_inputs(
        users, items, user_factors, item_factors, user_biases, item_biases
    )
    nc = build_nc(cap)
    res = run_bass_kernel_spmd(nc, in_maps, core_ids=list(range(NCORES)), trace=TRACE)
    LAST_RES = res
    out = np.zeros(B, np.float32)
    for c in range(NCORES):
        r = res.results[c]["out"]
        flat = r.T.reshape(-1)
        valid = el_of_slot[c] >= 0
        out[el_of_slot[c, valid]] = flat[valid]
    return out


# revision 3
# speedup vs baseline: 1.0267x; 1.0243x over previous
"""Biased MF batch scoring on 8 NeuronCores — async multi-queue dma_gather.

Key empirical rules (measured on this HW):
  - dma_gather on SWDGE queue 0 BLOCKS the GpSimd engine ~10ns/idx; on
    queues 1-3 it is fire-and-forget (~94ns dispatch), desc-gen runs on the
    queue's own Q7 cpu pair and the completion sem fires on DMA completion.
    => put all gathers on q1-q3, nothing on q0.
  - DVE rates: tensor_tensor ~0.6ns/elem/partition, tensor_reduce ~1.1.
    => slim item srows (4 x 66 f16 chunks in 768B) to cut compute 2x vs
    128-elem chunks, and chunk compute per bank block to overlap gathers.

Structure per core (elements assigned to cores by user-id range):
  - 4 user bank gathers (bank = 32768 ids, int16 idx), 256B rows, q3.
  - 4 item slot-chunk gathers from the k=4 packed slim table
    (25000 srows x 768B; srow j holds rows 4j..4j+3 as 66-f16 slices),
    alternating q1/q2. Slot order == user arrival order; the one-hot
    residue mask folds the 4-candidate dots into the final answer.
  - Vector: per block b: prod = irows*u(bcast), reduce 66 -> dots4;
    at the end: res = reduce_4(dots4 * mask).
"""

import numpy as np

GLOBAL_AVERAGE = 3.5
NU = 1_000_000
NI = 100_000
F = 64
B = 16384
NCORES = 8
USHARD = 131072
UBANK = 32768
NBANKS = 4
ISROWS = 25000
W = 128  # user row elements (fp16, 256B)
IW = 66  # slim item row elements
ISW = 4 * 96  # item srow elements (4*96 f16 = 768B; chunks at 66-elem strides padded)

TRACE = False
LAST_RES = None
_BUILD_CACHE = {}


def build_nc(cap, nbs):
    key = ("v11", cap, nbs)
    if key in _BUILD_CACHE:
        return _BUILD_CACHE[key]
    import concourse.bass as bass
    import concourse.mybir as mybir
    from concourse.ap import AP
    from concourse.library_config import mlp
    from contextlib import ExitStack

    S = NBANKS * cap
    C = S // 128
    cc = cap // 128

    nc = bass.Bass(num_swdge_queues=4)
    ucat = nc.dram_tensor("ucat", [USHARD, W], mybir.dt.float16, kind="ExternalInput")
    icat = nc.dram_tensor("icat", [ISROWS, ISW], mybir.dt.float16, kind="ExternalInput")
    uidx = nc.dram_tensor("uidx", [128, S // 16], mybir.dt.int16, kind="ExternalInput")
    iidx = nc.dram_tensor("iidx", [128, S // 16], mybir.dt.int16, kind="ExternalInput")
    mask = nc.dram_tensor("mask", [128, C * 4], mybir.dt.float32, kind="ExternalInput")
    didx = nc.dram_tensor("didx", [128, 1], mybir.dt.int16, kind="ExternalInput")
    out = nc.dram_tensor("out", [128, C], mybir.dt.float32, kind="ExternalOutput")

    with ExitStack() as stack:
        e = stack.enter_context
        t_uidx = e(nc.sbuf_tensor("t_uidx", [128, S // 16], mybir.dt.int16))
        t_iidx = e(nc.sbuf_tensor("t_iidx", [128, S // 16], mybir.dt.int16))
        t_mask = e(nc.sbuf_tensor("t_mask", [128, C * 4], mybir.dt.float32))
        t_didx = e(nc.sbuf_tensor("t_didx", [128, 1], mybir.dt.int16))
        dscr = e(nc.sbuf_tensor("dscr", [128, ISW], mybir.dt.float16))
        urows = e(nc.sbuf_tensor("urows", [128, C * W], mybir.dt.float16))
        irows = e(nc.sbuf_tensor("irows", [128, C * ISW], mybir.dt.float16))
        prod = e(nc.sbuf_tensor("prod", [128, C * 4 * IW], mybir.dt.float16))
        dots4 = e(nc.sbuf_tensor("dots4", [128, C * 4], mybir.dt.float32))
        resm = e(nc.sbuf_tensor("resm", [128, C * 4], mybir.dt.float32))
        res = e(nc.sbuf_tensor("res", [128, C], mybir.dt.float32))
        s_idx = e(nc.semaphore("s_idx"))
        s_m = e(nc.semaphore("s_m"))
        s_iblk = [e(nc.semaphore(f"s_ib{b}")) for b in range(NBANKS)]
        s_ublk = [e(nc.semaphore(f"s_ub{b}")) for b in range(NBANKS)]
        s_i3b = e(nc.semaphore("s_i3b"))
        s_d = e(nc.semaphore("s_d"))
        s_dg = e(nc.semaphore("s_dg"))
        s_v = e(nc.semaphore("s_v"))
        s_o = e(nc.semaphore("s_o"))
        block = e(nc.Block())

        @block.sync
        def _(sy):
            sy.dma_start(t_didx[:], didx[:]).then_inc(s_d, 16)
            sy.dma_start(t_iidx[:], iidx[:]).then_inc(s_idx, 16)
            sy.dma_start(t_uidx[:], uidx[:]).then_inc(s_idx, 16)
            sy.dma_start(t_mask[:], mask[:]).then_inc(s_m, 16)
            sy.wait_ge(s_v, 2 * NBANKS + 2)
            sy.dma_start(out[:], res[:]).then_inc(s_o, 16)
            sy.wait_ge(s_o, 16)

        @block.gpsimd
        def _(g):
            g.load_library(mlp)
            g.wait_ge(s_d, 16)
            # tiny gather: absorbs the post-reload dispatch block on its own
            # queue so the real calls dispatch fire-and-forget
            g.dma_gather(
                out_ap=dscr[:].rearrange("p (c k) -> p c k", k=ISW),
                in_ap=icat[:],
                idxs_ap=t_didx[:],
                num_idxs=16,
                num_idxs_reg=16,
                elem_size=ISW,
                queue_num=3,
                single_packet=False,
            ).then_inc(s_dg, 16)
            g.wait_ge(s_idx, 32)
            nvals = sorted(set(list(nbs) + [384, nbs[3] - 384]))
            regs = {n: g.to_reg(n) for n in nvals}
            iv = irows[:].rearrange("p (c k) -> p c k", k=ISW)
            uv = urows[:].rearrange("p (c k) -> p c k", k=W)
            def icall(b, q, lo=0, n=None, sem=None):
                n = nbs[b] - lo if n is None else n
                hi = b * cc + (lo + -(-n // 128) * 128) // 128
                g.dma_gather(
                    out_ap=iv[:, b * cc + lo // 128 : hi, :],
                    in_ap=icat[:],
                    idxs_ap=t_iidx[
                        :, b * (cap // 16) + lo // 16 : b * (cap // 16) + (lo + n) // 16
                    ],
                    num_idxs=n,
                    num_idxs_reg=regs[n],
                    elem_size=ISW,
                    queue_num=q,
                    single_packet=False,
                ).then_inc(sem if sem is not None else s_iblk[b], 16)

            def ucall(b, q):
                g.dma_gather(
                    out_ap=uv[:, b * cc : (b + 1) * cc, :],
                    in_ap=ucat[b * UBANK : (b + 1) * UBANK, :],
                    idxs_ap=t_uidx[
                        :, b * (cap // 16) : b * (cap // 16) + nbs[b] // 16
                    ],
                    num_idxs=nbs[b],
                    num_idxs_reg=regs[nbs[b]],
                    elem_size=W,
                    queue_num=q,
                    single_packet=False,
                ).then_inc(s_ublk[b], 16)

            # block-major pairs so each compute block completes ASAP;
            # queues chosen so consecutive calls never share a queue and
            # per-queue load is 3/3/2 (dummy occupies q3 first)
            icall(0, 1)
            ucall(0, 2)
            icall(1, 3)
            ucall(1, 1)
            icall(2, 2)
            ucall(2, 3)
            icall(3, 1, n=384)
            icall(3, 3, lo=384, sem=s_i3b)
            ucall(3, 2)

        @block.vector
        def _(vec):
            nv = 0
            for b in range(NBANKS):
                vec.wait_ge(s_iblk[b], 16)
                if b == 3:
                    vec.wait_ge(s_i3b, 16)
                vec.wait_ge(s_ublk[b], 16)
                # prod[b] = irows chunks (4 x 66) * u (bcast over 4)
                ivb = irows[:].rearrange("p (c k) -> p c k", k=ISW)[
                    :, b * cc : (b + 1) * cc, :
                ]
                i4 = AP(
                    ivb.tensor,
                    ivb.offset,
                    [list(ivb.ap[0]), list(ivb.ap[1]), [96, 4], [1, IW]],
                )
                uvb = urows[:].rearrange("p (c k) -> p c k", k=W)[
                    :, b * cc : (b + 1) * cc, :
                ]
                u4 = AP(
                    uvb.tensor,
                    uvb.offset,
                    [list(uvb.ap[0]), list(uvb.ap[1]), [0, 4], [1, IW]],
                )
                p4 = prod[:].rearrange("p (c r k) -> p c r k", r=4, k=IW)[
                    :, b * cc : (b + 1) * cc, :, :
                ]
                vec.tensor_mul(p4, i4, u4).then_inc(s_v, 1)
                nv += 1
                vec.wait_ge(s_v, nv)
                vec.reduce_sum(
                    dots4[:, b * cc * 4 * 128 // 128 : (b + 1) * cc * 4 * 128 // 128],
                    prod[:, b * cc * 4 * IW * 128 // 128 : (b + 1) * cc * 4 * IW * 128 // 128]
                    .rearrange("p (g k) -> p g k", k=IW),
                    axis=mybir.AxisListType.X,
                ).then_inc(s_v, 1)
                nv += 1
                vec.wait_ge(s_v, nv)
            vec.wait_ge(s_m, 16)
            vec.tensor_mul(resm[:], dots4[:], t_mask[:]).then_inc(s_v, 1)
            nv += 1
            vec.wait_ge(s_v, nv)
            vec.reduce_sum(
                res[:],
                resm[:].rearrange("p (c r) -> p c r", r=4),
                axis=mybir.AxisListType.X,
            ).then_inc(s_v, 1)

    nc.finalize()
    mybir.codegen_inst_isa_subclasses(nc)
    _BUILD_CACHE[key] = nc
    return nc


def _wrap16(vals):
    n = len(vals)
    assert n % 16 == 0
    w = np.asarray(vals, np.int16).reshape(n // 16, 16).T
    return np.tile(w, (8, 1))


def make_inputs(users, items, user_factors, item_factors, user_biases, item_biases):
    users = np.asarray(users, np.int64)
    items = np.asarray(items, np.int64)

    ucat = np.zeros((NCORES * USHARD, W), np.float16)
    ucat[:NU, :F] = user_factors
    ucat[:NU, F] = np.asarray(user_biases, np.float32).reshape(NU)
    ucat[:NU, F + 1] = 1.0
    # slim item rows: [itf(64) | 1 | ib+3.5] = 66 f16, packed 4/srow at 96-elem strides
    irow = np.zeros((NI, IW), np.float16)
    irow[:, :F] = item_factors
    irow[:, F] = 1.0
    irow[:, F + 1] = np.asarray(item_biases, np.float32).reshape(NI) + np.float32(
        GLOBAL_AVERAGE
    )
    icat = np.zeros((ISROWS, ISW), np.float16)
    for r in range(4):
        icat[:, r * 96 : r * 96 + IW] = irow[r::4]
    ushards = ucat.reshape(NCORES, USHARD, W)

    core_of = users >> 17
    bank_of = (users >> 15) & 3
    els = [
        [np.nonzero((core_of == c) & (bank_of == b))[0] for b in range(NBANKS)]
        for c in range(NCORES)
    ]
    maxn = max(len(e) for per in els for e in per)
    cap = -(-maxn // 128) * 128
    nbs = tuple(
        -(-max(len(els[c][b]) for c in range(NCORES)) // 16) * 16
        for b in range(NBANKS)
    )
    # the i3 split needs nbs[3] > 384 and every bank nonempty-capped
    nbs = tuple(max(n, 16) for n in nbs)
    if nbs[3] <= 384:
        nbs = nbs[:3] + (400,)
    S = NBANKS * cap
    C = S // 128

    in_maps = []
    el_of_slot = np.full((NCORES, S), -1, np.int64)
    for c in range(NCORES):
        uidx = np.zeros(S, np.int16)
        iidx = np.zeros(S, np.int16)
        msk = np.zeros((S, 4), np.float32)
        for b in range(NBANKS):
            e = els[c][b]
            n = len(e)
            base = b * cap
            el_of_slot[c, base : base + n] = e
            uidx[base : base + n] = (users[e] & 32767).astype(np.int16)
            iidx[base : base + n] = (items[e] >> 2).astype(np.int16)
            msk[base + np.arange(n), (items[e] & 3)] = 1.0
        uidx_w = np.concatenate(
            [_wrap16(uidx[b * cap : (b + 1) * cap]) for b in range(NBANKS)], axis=1
        )
        iidx_w = np.concatenate(
            [_wrap16(iidx[b * cap : (b + 1) * cap]) for b in range(NBANKS)], axis=1
        )
        mask_t = np.ascontiguousarray(
            msk.reshape(C, 128, 4).transpose(1, 0, 2).reshape(128, C * 4)
        )
        in_maps.append(
            {
                "ucat": ushards[c],
                "icat": icat,
                "uidx": np.ascontiguousarray(uidx_w),
                "iidx": np.ascontiguousarray(iidx_w),
                "mask": mask_t,
                "didx": np.zeros((128, 1), np.int16),
            }
        )
    return cap, nbs, in_maps, el_of_slot


def kernel(users, items, user_factors, item_factors, user_biases, item_biases):
    global LAST_RES
    from concourse.bass_utils import run_bass_kernel_spmd

    cap, nbs, in_maps, el_of_slot = make_inputs(
        users, items, user_factors, item_factors, user_biases, item_biases
    )
    nc = build_nc(cap, nbs)
    res = run_bass_kernel_spmd(nc, in_maps, core_ids=list(range(NCORES)), trace=TRACE)
    LAST_RES = res
    out = np.zeros(B, np.float32)
    for c in range(NCORES):
        r = res.results[c]["out"]
        flat = r.T.reshape(-1)
        valid = el_of_slot[c] >= 0
        out[el_of_slot[c, valid]] = flat[valid]
    return out
